# revision 2
# baseline (speedup 1.0000x reference)
"""Multi-head self-attention (CrossAttention with encoder_hidden_states=None)
on 8 Trainium2 NeuronCores.

Problem: hidden_states [B=4, S=2048, D=512], 8 heads x 64 dim, fp32 in/out.
    q/k/v = x @ W{q,k,v};  per-head softmax(q k^T / 8) v;  out proj + bias.

Sharding: core c = (batch b = c//2, query-half qh = c%2) handles a
1024-query slice of one batch element; K/V cover the full 2048 tokens.
Outputs are disjoint slices -> pure concatenation, no output comm.

The expensive path on this rig is host<->device bytes, so uploads are
minimized and deduplicated (all bf16; numpy-simulated accuracy of the
full bf16 pipeline: rel err ~5e-3 vs the 2e-2 gate):

    xh [512, 1024] bf16 (1MB)  - the core's OWN token half, transposed
    ws [512, 256]  bf16 (256KB) - 1/8 column shard of [Wq|Wk|Wv|Wo]
    bo [512] f32; out [1024, 512] bf16 (1MB) download

On-device, an AllGather over each batch pair rebuilds the full 2048-token
activation, and an AllGather over all 8 cores rebuilds the 2MB weight
block (collectives bounce through internal DRAM; gathered blocks are
[comm, rows, cols] and are read to SBUF with strided APs). A core's
queries are its own uploaded half, so the query slice needs no gather and
no per-core control flow.

Compute dataflow (feature dim on partitions throughout; fp32 PSUM):
    QT[d, q] = Wq^T xq^T         KT[d, k] = Wk^T x^T
    V[k, d]  = x Wv              (+1s column appended per head)
    S^T[k, q] = (KT_h)^T QT_h    (64-deep contraction; the 2 heads of a
                                  pair use disjoint PE row groups)
    P^T = exp(S^T / 8)           (ACT, unnormalized, bf16 out)
    O^T[dh+1, q] = [V_h | 1]^T P^T   (1s row -> softmax denominators)
    AoT[d, q] = O^T rows * 1/denom   (gpsimd broadcast + DVE mul)
    out[t, d] = AoT^T Wo + bo        (bf16 store, fp32 upcast on host)

If the collective path fails in the grading environment, kernel() falls
back to a collective-free variant (replicated full-x/full-w uploads).
"""

import numpy as np
import ml_dtypes

import concourse.bass as bass
import concourse.mybir as mybir
import concourse.tile as tile
from concourse import bacc
from concourse.bass_utils import run_bass_kernel_spmd
from contextlib import ExitStack

F32 = mybir.dt.float32
BF16 = mybir.dt.bfloat16

B, S, D = 4, 2048, 512
H, DH = 8, 64
SCALE = DH ** -0.5  # 0.125
NCORES = 8
QS = S // 2    # query tokens per core (1024)
KC = D // 128  # feature chunks (4)
TC = S // 128  # token chunks (16)
WS = 4 * D // NCORES  # weight shard columns (256)

_CACHE = {}
LAST_RESULTS = None


def _emit_compute(nc, tc, ctx, xq_src, x_src, w_src, bo, out):
    """Shared compute body. xq_src/x_src/w_src are callables returning the
    DRAM APs to load (queries [D, QS], full tokens [D, S]-equivalent via
    3D AP, weights [D, 4D]-equivalent)."""
    WQ, WK, WV, WO = 0, D, 2 * D, 3 * D

    xp = ctx.enter_context(tc.tile_pool(name="xp", bufs=4))
    xqp = ctx.enter_context(tc.tile_pool(name="xqp", bufs=4))
    wp = ctx.enter_context(tc.tile_pool(name="wp", bufs=4))
    qtp = ctx.enter_context(tc.tile_pool(name="qtp", bufs=4))
    ktp = ctx.enter_context(tc.tile_pool(name="ktp", bufs=4))
    vap = ctx.enter_context(tc.tile_pool(name="vap", bufs=16))
    ptp = ctx.enter_context(tc.tile_pool(name="ptp", bufs=3))
    aop = ctx.enter_context(tc.tile_pool(name="aop", bufs=4))
    ost = ctx.enter_context(tc.tile_pool(name="ost", bufs=2))
    rbp = ctx.enter_context(tc.tile_pool(name="rbp", bufs=2))
    rcp = ctx.enter_context(tc.tile_pool(name="rcp", bufs=2))
    singles = ctx.enter_context(tc.tile_pool(name="singles", bufs=1))
    psA = ctx.enter_context(tc.tile_pool(name="psA", bufs=2, space="PSUM"))
    psO = ctx.enter_context(tc.tile_pool(name="psO", bufs=2, space="PSUM"))

    # ---- SBUF loads -----------------------------------------------------
    xq_t = []
    for kc in range(KC):
        t = xqp.tile([128, QS], BF16, tag="xqp")
        nc.sync.dma_start(out=t, in_=xq_src(kc))
        xq_t.append(t)
    w_t = []
    for kc in range(KC):
        t = wp.tile([128, 4 * D], BF16, tag="wp")
        nc.sync.dma_start(out=t, in_=w_src(kc))
        w_t.append(t)
    x_t = []
    for kc in range(KC):
        t = xp.tile([128, S], BF16, tag="xp")
        nc.sync.dma_start(out=t, in_=x_src(kc))
        x_t.append(t)

    bo_b = singles.tile([128, D], F32)
    bo_bcast_ap = bass.AP(tensor=bo.tensor, offset=bo.offset,
                          ap=[[0, 128]] + list(bo.ap))
    nc.sync.dma_start(out=bo_b, in_=bo_bcast_ap)
    ones_h = singles.tile([128, H, 1], BF16)
    nc.vector.memset(ones_h, 1.0)

    # ---- QT[d, q] = Wq^T @ xq^T  (4 tiles [128, QS] bf16) ---------------
    qt = []
    for dc in range(KC):
        ps = psA.tile([128, QS], F32, tag="psA")
        for kc in range(KC):
            lhsT = w_t[kc][:, WQ + dc * 128:WQ + (dc + 1) * 128]
            for nh in range(QS // 512):
                nc.tensor.matmul(
                    ps[:, nh * 512:(nh + 1) * 512], lhsT,
                    xq_t[kc][:, nh * 512:(nh + 1) * 512],
                    start=(kc == 0), stop=(kc == KC - 1))
        t = qtp.tile([128, QS], BF16, tag="qtp")
        nc.vector.tensor_copy(out=t, in_=ps)
        qt.append(t)

    # ---- KT[d, k] = Wk^T @ x^T  (4 tiles [128, S] bf16) -----------------
    kt = []
    for dc in range(KC):
        t = ktp.tile([128, S], BF16, tag="ktp", name="kt")
        for half in range(2):
            ps = psA.tile([128, 1024], F32, tag="psA", name="ps")
            for kc in range(KC):
                lhsT = w_t[kc][:, WK + dc * 128:WK + (dc + 1) * 128]
                for nh in range(2):
                    col = half * 1024 + nh * 512
                    nc.tensor.matmul(
                        ps[:, nh * 512:(nh + 1) * 512], lhsT,
                        x_t[kc][:, col:col + 512],
                        start=(kc == 0), stop=(kc == KC - 1))
            nc.vector.tensor_copy(
                out=t[:, half * 1024:(half + 1) * 1024], in_=ps)
        kt.append(t)

    # ---- V_aug[k, h, 0:64]=x@Wv slice, [..,64]=1  (16 tiles) ------------
    va = []
    for tci in range(TC):
        ps = psO.tile([128, 512], F32, tag="psO")
        for kc in range(KC):
            nc.tensor.matmul(
                ps, x_t[kc][:, tci * 128:(tci + 1) * 128],
                w_t[kc][:, WV:WV + D],
                start=(kc == 0), stop=(kc == KC - 1))
        t = vap.tile([128, H, DH + 1], BF16, tag="vap")
        nc.vector.tensor_copy(
            out=t[:, :, 0:DH],
            in_=ps.rearrange("p (h d) -> p h d", h=H))
        nc.vector.tensor_copy(out=t[:, :, DH:DH + 1], in_=ones_h)
        va.append(t)

    # ---- attention; AoT[d, q] tiles [128, QS] bf16 ----------------------
    aot = [aop.tile([128, QS], BF16, tag="aop", name="aot")
           for _ in range(H // 2)]

    for hp in range(H // 2):
        pso = [psO.tile([DH + 1, QS], F32, tag="psO", name="pso")
               for _ in range(2)]
        for tci in range(TC):
            pss = [psA.tile([128, QS], F32, tag="psA", name="pss")
                   for _ in range(2)]
            for hh in range(2):
                r0 = hh * DH
                for j in range(2):
                    nc.tensor.matmul(
                        pss[hh][:, j * 512:(j + 1) * 512],
                        kt[hp][r0:r0 + DH, tci * 128:(tci + 1) * 128],
                        qt[hp][r0:r0 + DH, j * 512:(j + 1) * 512],
                        start=True, stop=True)
            for hh in range(2):
                pt = ptp.tile([128, QS], BF16, tag="ptp")
                nc.scalar.activation(
                    out=pt, in_=pss[hh],
                    func=mybir.ActivationFunctionType.Exp, scale=SCALE)
                h = hp * 2 + hh
                for j in range(2):
                    nc.tensor.matmul(
                        pso[hh][:, j * 512:(j + 1) * 512],
                        va[tci][:, h, :], pt[:, j * 512:(j + 1) * 512],
                        start=(tci == 0), stop=(tci == TC - 1))
        for hh in range(2):
            rc = rcp.tile([1, QS], F32, tag="rcp")
            nc.vector.reciprocal(rc, pso[hh][DH:DH + 1, :])
            rb = rbp.tile([DH, QS], F32, tag="rbp")
            nc.gpsimd.partition_broadcast(rb, rc)
            nc.vector.tensor_mul(
                aot[hp][hh * DH:(hh + 1) * DH, :],
                pso[hh][0:DH, :], rb)

    # ---- out[t, d] = AoT^T @ Wo + bo, bf16 store ------------------------
    for tci in range(QS // 128):
        ps = psO.tile([128, 512], F32, tag="psO")
        for dc in range(KC):
            nc.tensor.matmul(
                ps, aot[dc][:, tci * 128:(tci + 1) * 128],
                w_t[dc][:, WO:WO + D],
                start=(dc == 0), stop=(dc == KC - 1))
        ot = ost.tile([128, D], BF16, tag="ost")
        nc.vector.tensor_add(ot, ps, bo_b)
        nc.sync.dma_start(out=out[tci * 128:(tci + 1) * 128, :], in_=ot)


def _build_gather():
    """Sharded-upload variant: x-half + weight shard in, AllGathers on
    device."""
    nc = bacc.Bacc("TRN2", target_bir_lowering=False, debug=False,
                   enable_asserts=False, num_devices=NCORES)

    xh = nc.dram_tensor("xh", [D, QS], BF16, kind="ExternalInput").ap()
    ws = nc.dram_tensor("ws", [D, WS], BF16, kind="ExternalInput").ap()
    bo = nc.dram_tensor("bo", [D], F32, kind="ExternalInput").ap()
    out = nc.dram_tensor("out", [QS, D], BF16, kind="ExternalOutput").ap()
    # internal bounce + gather targets (collectives cannot touch IO tensors)
    xhi = nc.dram_tensor("xhi", [D, QS], BF16).ap()
    wsi = nc.dram_tensor("wsi", [D, WS], BF16).ap()
    xg = nc.dram_tensor("xg", [2, D, QS], BF16).ap()
    wg = nc.dram_tensor("wg", [NCORES, D, WS], BF16, addr_space="Shared").ap()

    with tile.TileContext(nc) as tc, ExitStack() as ctx:
        nc.sync.dma_start(out=wsi, in_=ws)
        nc.sync.dma_start(out=xhi, in_=xh)
        nc.gpsimd.collective_compute(
            "AllGather", mybir.AluOpType.bypass,
            replica_groups=[[i for i in range(NCORES)]],
            ins=[wsi], outs=[wg])
        nc.gpsimd.collective_compute(
            "AllGather", mybir.AluOpType.bypass,
            replica_groups=[[2 * p, 2 * p + 1] for p in range(NCORES // 2)],
            ins=[xhi], outs=[xg])

        def xq_src(kc):
            return xh[kc * 128:(kc + 1) * 128, :]

        def w_src(kc):
            # [128, 8, 256] slice over the 8 gathered shards -> [128, 2048]
            return bass.AP(tensor=wg.tensor, offset=wg.offset + kc * 128 * WS,
                           ap=[[WS, 128], [D * WS, NCORES], [1, WS]])

        def x_src(kc):
            # [128, 2, 1024] slice over the 2 gathered halves -> [128, 2048]
            return bass.AP(tensor=xg.tensor, offset=xg.offset + kc * 128 * QS,
                           ap=[[QS, 128], [D * QS, 2], [1, QS]])

        _emit_compute(nc, tc, ctx, xq_src, x_src, w_src, bo, out)

    nc.compile()
    return nc


def _build_replicated():
    """Fallback without collectives: full x (query half first) + full w."""
    nc = bacc.Bacc("TRN2", target_bir_lowering=False, debug=False,
                   enable_asserts=False)

    x = nc.dram_tensor("x", [D, S], BF16, kind="ExternalInput").ap()
    w = nc.dram_tensor("w", [D, 4 * D], BF16, kind="ExternalInput").ap()
    bo = nc.dram_tensor("bo", [D], F32, kind="ExternalInput").ap()
    out = nc.dram_tensor("out", [QS, D], BF16, kind="ExternalOutput").ap()

    with tile.TileContext(nc) as tc, ExitStack() as ctx:
        _emit_compute(
            nc, tc, ctx,
            xq_src=lambda kc: x[kc * 128:(kc + 1) * 128, 0:QS],
            x_src=lambda kc: x[kc * 128:(kc + 1) * 128, :],
            w_src=lambda kc: w[kc * 128:(kc + 1) * 128, :],
            bo=bo, out=out)

    nc.compile()
    return nc


def _prep_host(hidden_states, Wq, Wk, Wv, Wo, bo):
    hidden_states = np.asarray(hidden_states, dtype=np.float32)
    w_cat = np.concatenate(
        [np.asarray(a, dtype=np.float32) for a in (Wq, Wk, Wv, Wo)],
        axis=1).astype(ml_dtypes.bfloat16)
    bo = np.asarray(bo, dtype=np.float32)
    xT = [np.ascontiguousarray(hidden_states[b].T).astype(ml_dtypes.bfloat16)
          for b in range(B)]
    return xT, w_cat, bo


def _run_gather(xT, w_cat, bo):
    if "nc_g" not in _CACHE:
        _CACHE["nc_g"] = _build_gather()
    nc = _CACHE["nc_g"]
    in_maps = []
    for c in range(NCORES):
        b, qh = c // 2, c % 2
        in_maps.append({
            "xh": np.ascontiguousarray(xT[b][:, qh * QS:(qh + 1) * QS]),
            "ws": np.ascontiguousarray(w_cat[:, c * WS:(c + 1) * WS]),
            "bo": bo,
        })
    return run_bass_kernel_spmd(nc, in_maps, core_ids=list(range(NCORES)))


def _run_replicated(xT, w_cat, bo):
    if "nc_r" not in _CACHE:
        _CACHE["nc_r"] = _build_replicated()
    nc = _CACHE["nc_r"]
    # odd cores: token halves swapped so queries are always the first QS
    # columns (attention is permutation-invariant over keys; K and V permute
    # together)
    xT_sw = [np.ascontiguousarray(
        np.concatenate([t[:, QS:], t[:, :QS]], axis=1)) for t in xT]
    in_maps = []
    for c in range(NCORES):
        b, qh = c // 2, c % 2
        in_maps.append({
            "x": xT[b] if qh == 0 else xT_sw[b],
            "w": w_cat, "bo": bo,
        })
    return run_bass_kernel_spmd(nc, in_maps, core_ids=list(range(NCORES)))


def kernel(hidden_states, Wq, Wk, Wv, Wo, bo):
    global LAST_RESULTS
    xT, w_cat, bo = _prep_host(hidden_states, Wq, Wk, Wv, Wo, bo)

    if _CACHE.get("no_collectives"):
        res = _run_replicated(xT, w_cat, bo)
    else:
        try:
            res = _run_gather(xT, w_cat, bo)
        except Exception:
            _CACHE["no_collectives"] = True
            res = _run_replicated(xT, w_cat, bo)
    LAST_RESULTS = res

    out = np.empty((B, S, D), dtype=np.float32)
    for c in range(NCORES):
        b, qh = c // 2, c % 2
        out[b, qh * QS:(qh + 1) * QS, :] = res.results[c]["out"].astype(
            np.float32)
    return out
